# revision 18
# baseline (speedup 1.0000x reference)
"""Longformer self-attention Bass kernel for 8 Trainium2 NeuronCores.

Sharding: data-parallel over batch (2) x sequence-parallel (4 chunks of 1024
queries). Each core receives a transposed x slab covering its rows +-W halo
(zero padded at sequence edges), the 64 global rows, and the full weight set.

On-device layout choices:
  - "T layout" [E_out on partitions, seq on free] for q/k/kg/qg so the banded
    score matmuls need no transposes; head pairs share 128-partition slices.
  - Scores are computed keys-on-partitions (S^T). Softmax runs without max
    subtraction (scores are O(1)); the denominator comes for free from a
    ones-column appended to v, so P@V and sum(exp) are one PSUM accumulation.
  - The global keys are zero-padded to 128 so their scores live in the same
    PSUM tile as the band ([128, 768] per head: 5 band chunks + glob block)
    and one Exp covers everything; padded keys get exp(0)=1 but multiply
    zeroed v/ones columns, contributing nothing.
  - Band-edge masking: score chunk order is [c0, c4, c1, c2, c3] so the two
    masked chunks are adjacent and one [128,256] mask multiply per head
    handles the window geometry.
  - Software pipelining: scores for head-pair p+1 are emitted before the PV
    matmuls of pair p, hiding the exp/mask latency behind PE work.
  - Per query tile, all 12 heads' PV accumulate into one PSUM tile
    [128, H, 65]; one strided reciprocal per tile replaces 12, and the
    normalize scale-muls run on the otherwise idle GpSimd engine.
  - All masking is data-driven (uniform SPMD program): key validity (padding
    + global-key exclusion from the band) is folded into the v/ones columns.
"""

import sys
import numpy as np

if '/opt/trn_rl_repo' not in sys.path:
    sys.path.insert(0, '/opt/trn_rl_repo')

import ml_dtypes

B, S, E, H, HD, G, W = 2, 4096, 768, 12, 64, 64, 256
NCORES = 8
CHUNK = 1024          # query rows per core
SLAB = 1536           # halo slab rows per core (CHUNK + 2W)
NQT = CHUNK // 128    # query tiles per core
NSC = SLAB // 128     # slab chunks of 128 keys
NGC = CHUNK // 128    # key chunks for the global-token partials
KE = E // 128         # contraction chunks over E
VW = HD + 1           # v head width incl. ones column
GP = 128              # global keys padded to 128
BF16 = ml_dtypes.bfloat16

# score-tile column order: key chunks [0, 4, 1, 2, 3]; masked chunks first.
PERM = (0, 4, 1, 2, 3)
# PV consumption order: unmasked middle chunks, then glob, then masked edges
PV_ORDER = (2, 3, 4, -1, 0, 1)   # -1 = glob block (cols 640:768)

_PROGRAM = None


def _build_program():
    import concourse.bass as bass
    import concourse.mybir as mybir
    import concourse.tile as tile
    from concourse import bacc

    dt = mybir.dt
    Act = mybir.ActivationFunctionType

    nc = bacc.Bacc("TRN2", target_bir_lowering=False, debug=False,
                   num_devices=NCORES)

    xT = nc.dram_tensor("xT", [128, KE, SLAB], dt.bfloat16, kind="ExternalInput")
    xTg = nc.dram_tensor("xTg", [128, KE, GP], dt.bfloat16, kind="ExternalInput")
    wts = {}
    for nm in ("wqt", "wkt", "wvt", "wkgt", "wvgt", "wqgt"):
        wts[nm] = nc.dram_tensor(nm, [128, KE, E], dt.bfloat16, kind="ExternalInput")
    vmask_d = nc.dram_tensor("vmask", [128, NSC], dt.float32, kind="ExternalInput")
    vones_d = nc.dram_tensor("vones", [128, NSC, H], dt.bfloat16, kind="ExternalInput")
    gmask_d = nc.dram_tensor("gmask", [G, 1], dt.float32, kind="ExternalInput")
    gones_d = nc.dram_tensor("gones", [G, H], dt.bfloat16, kind="ExternalInput")
    mask2_d = nc.dram_tensor("mask2", [128, 256], dt.bfloat16, kind="ExternalInput")
    out_band = nc.dram_tensor("out_band", [CHUNK, E], dt.float32, kind="ExternalOutput")
    gpart = nc.dram_tensor("gpart", [G, H, VW], dt.float32, kind="ExternalOutput")

    with tile.TileContext(nc) as tc:
        with tc.tile_pool(name="main", bufs=1) as mp, \
             tc.tile_pool(name="psum", bufs=1, space="PSUM") as pp:

            # ---- resident SBUF tensors -------------------------------------
            xT_sb = mp.tile([128, KE, SLAB], dt.bfloat16)
            xTg_sb = mp.tile([128, KE, GP], dt.bfloat16)
            w_sb = {nm: mp.tile([128, KE, E], dt.bfloat16, name=f"w_{nm}")
                    for nm in wts}
            qTzA = mp.tile([128, KE, CHUNK], dt.bfloat16, name="qTzA")
            qTzB = mp.tile([128, KE, CHUNK], dt.bfloat16, name="qTzB")
            kT = mp.tile([128, KE, SLAB], dt.bfloat16)
            kgT = mp.tile([128, KE, CHUNK], dt.bfloat16)
            kglobT = mp.tile([128, KE, GP], dt.bfloat16)
            qgT = mp.tile([128, KE, G], dt.bfloat16)
            vext = mp.tile([128, NSC, H * VW], dt.bfloat16)
            vgext = mp.tile([128, NGC, H * VW], dt.bfloat16)
            vglob = mp.tile([GP, H * VW], dt.bfloat16)
            vmask = mp.tile([128, NSC], dt.float32)
            gmask = mp.tile([G, 1], dt.float32)
            mask2 = mp.tile([128, 256], dt.bfloat16)

            nc.sync.dma_start(xT_sb[:, :, :], xT[:, :, :])
            nc.sync.dma_start(xTg_sb[:, :, :], xTg[:, :, :])
            for nm in wts:
                nc.sync.dma_start(w_sb[nm][:, :, :], wts[nm][:, :, :])
            nc.sync.dma_start(vmask[:, :], vmask_d[:, :])
            nc.sync.dma_start(gmask[:, :], gmask_d[:, :])
            nc.sync.dma_start(mask2[:, :], mask2_d[:, :])
            # vglob rows G:GP stay zero (padded global keys kill their own
            # exp(0)=1 contributions); zero everything, then fill rows 0:G.
            nc.vector.memset(vglob[:, :], 0.0)
            nc.gpsimd.memset(qTzA[64:128, :, :], 0.0)
            nc.gpsimd.memset(qTzB[0:64, :, :], 0.0)
            # ones columns (gated by key validity)
            nc.sync.dma_start(
                vext.rearrange("p c (h e) -> p c h e", e=VW)[:, :, :, HD:HD + 1],
                vones_d.rearrange("p c (h o) -> p c h o", o=1))
            nc.sync.dma_start(
                vglob.rearrange("p (h e) -> p h e", e=VW)[0:G, :, HD:HD + 1],
                gones_d.rearrange("p (h o) -> p h o", o=1))
            nc.vector.memset(
                vgext.rearrange("p c (h e) -> p c h e", e=VW)[:, :, :, HD:HD + 1], 1.0)

            # ---- phase 1: projections --------------------------------------
            def proj_T(dst, wname, src, src_off, n_total, bias=None):
                # dst[e_out, n] = sum_e w[e, e_out] * src[e, src_off + n]
                for m in range(KE):
                    n0 = 0
                    while n0 < n_total:
                        nw = min(512, n_total - n0)
                        ps = pp.tile([128, 512], dt.float32, tag="sc", bufs=2,
                                     name="ps_projT")
                        for k in range(KE):
                            nc.tensor.matmul(
                                ps[:, :nw],
                                w_sb[wname][:, k, m * 128:(m + 1) * 128],
                                src[:, k, src_off + n0: src_off + n0 + nw],
                                start=(k == 0), stop=(k == KE - 1))
                        nc.scalar.activation(dst[:, m, n0:n0 + nw], ps[:, :nw],
                                             Act.Copy)
                        n0 += nw

            def proj_q():
                # like proj_T but splits each 128-row block between the two
                # zero-padded q tensors (head 2m rows 0:64 -> qTzA, head 2m+1
                # rows 64:128 -> qTzB) so score matmuls of a head pair can
                # share one full-128-row kT stationary
                for m in range(KE):
                    n0 = 0
                    while n0 < CHUNK:
                        nw = min(512, CHUNK - n0)
                        ps = pp.tile([128, 512], dt.float32, tag="sc", bufs=2,
                                     name="ps_projq")
                        for k in range(KE):
                            nc.tensor.matmul(
                                ps[:, :nw],
                                w_sb["wqt"][:, k, m * 128:(m + 1) * 128],
                                xT_sb[:, k, W + n0: W + n0 + nw],
                                start=(k == 0), stop=(k == KE - 1))
                        nc.scalar.activation(qTzA[0:64, m, n0:n0 + nw],
                                             ps[0:64, :nw], Act.Copy)
                        nc.scalar.activation(qTzB[64:128, m, n0:n0 + nw],
                                             ps[64:128, :nw], Act.Copy)
                        n0 += nw

            proj_q()
            proj_T(kT, "wkt", xT_sb, 0, SLAB)
            proj_T(kglobT, "wkt", xTg_sb, 0, GP)
            proj_T(kgT, "wkgt", xT_sb, W, CHUNK)
            proj_T(qgT, "wqgt", xTg_sb, 0, G)

            def proj_V(dst, wname, src, src_off, n_chunks, mask_tile):
                # dst[seq, h, d] = sum_e src[e, seq] * w[e, 64h+d], * mask[seq]
                for sI in range(n_chunks):
                    for h0, nw in ((0, 512), (512, 256)):
                        ps = pp.tile([128, 512], dt.float32, tag="sc", bufs=2,
                                     name="ps_projV")
                        for k in range(KE):
                            nc.tensor.matmul(
                                ps[:, :nw],
                                src[:, k, src_off + sI * 128: src_off + (sI + 1) * 128],
                                w_sb[wname][:, k, h0:h0 + nw],
                                start=(k == 0), stop=(k == KE - 1))
                        dv = dst.rearrange("p c (h e) -> p c h e", e=VW)[
                            :, sI, h0 // HD:(h0 + nw) // HD, 0:HD]
                        pv = ps[:, :nw].rearrange("p (h e) -> p h e", e=HD)
                        if mask_tile is None:
                            nc.vector.tensor_copy(dv, pv)
                        else:
                            nc.vector.tensor_scalar_mul(dv, pv,
                                                        mask_tile[:, sI:sI + 1])

            # vext v-data is gated by per-key validity (padding + global-key
            # exclusion from the band); vgext is not masked (global-token
            # attention sees every key).
            proj_V(vext, "wvt", xT_sb, 0, NSC, vmask)
            proj_V(vgext, "wvgt", xT_sb, W, NGC, None)

            # v of the global rows (for the global-key part of the band
            # softmax), gated by j < n_global
            for h0, nw in ((0, 512), (512, 256)):
                psg = pp.tile([128, 512], dt.float32, tag="sc", bufs=2,
                              name="ps_vglob")
                for k in range(KE):
                    nc.tensor.matmul(
                        psg[0:G, :nw], xTg_sb[:, k, 0:G], w_sb["wvt"][:, k, h0:h0 + nw],
                        start=(k == 0), stop=(k == KE - 1))
                dv = vglob.rearrange("p (h e) -> p h e", e=VW)[
                    0:G, h0 // HD:(h0 + nw) // HD, 0:HD]
                nc.vector.tensor_scalar_mul(
                    dv, psg[0:G, :nw].rearrange("p (h e) -> p h e", e=HD),
                    gmask[:, :])

            # ---- phase 2: banded + global-key attention --------------------
            # Per head: one [128, 768] PSUM score tile (5 band chunks in PERM
            # order + padded glob block), one Exp, one [128,256] edge mask.
            # Scores of pair p+1 are emitted before the PV of pair p so the
            # exp/mask chain hides behind PE work.
            pv_tiles = {}      # t -> psum accumulator [128, H, VW]
            attn_tiles = {}

            def emit_scores(t, p):
                hp = p
                qsA = qTzA[:, hp, t * 128:(t + 1) * 128]
                qsB = qTzB[:, hp, t * 128:(t + 1) * 128]
                # one PSUM tile for the pair (A: cols 0:768, B: 768:1536) so
                # a single Exp covers both heads.  Full-128-row stationary
                # (both heads' k dims); the moving q is zero-padded on the
                # other head's rows, so each head's scores are exact and
                # consecutive A/B matmuls reuse the loaded weights.
                scp = pp.tile([128, 1536], dt.float32, tag="sc", bufs=2,
                              name="ps_scp")
                for ci, src in enumerate(PERM):
                    kst = kT[:, hp, (t + src) * 128:(t + src + 1) * 128]
                    nc.tensor.matmul(scp[:, ci * 128:(ci + 1) * 128], kst,
                                     qsA, start=True, stop=True)
                    nc.tensor.matmul(scp[:, 768 + ci * 128:768 + (ci + 1) * 128],
                                     kst, qsB, start=True, stop=True)
                nc.tensor.matmul(scp[:, 640:768], kglobT[:, hp, :],
                                 qsA, start=True, stop=True)
                nc.tensor.matmul(scp[:, 1408:1536], kglobT[:, hp, :],
                                 qsB, start=True, stop=True)
                et = mp.tile([128, 1536], dt.bfloat16, tag="et", bufs=3,
                             name="et")
                # two exps (A-half first): halves the latency until the PV
                # matmuls of head A can start
                nc.scalar.activation(et[:, 0:768], scp[:, 0:768], Act.Exp)
                nc.vector.tensor_mul(et[:, 0:256], et[:, 0:256], mask2[:, :])
                nc.scalar.activation(et[:, 768:1536], scp[:, 768:1536], Act.Exp)
                nc.vector.tensor_mul(et[:, 768:1024], et[:, 768:1024],
                                     mask2[:, :])
                return et

            def emit_pv(t, p, et):
                # two 6-head accumulators: 6*65 floats = 390 <= 512 keeps every
                # per-head matmul output inside one PSUM bank
                if t not in pv_tiles:
                    pv_tiles[t] = (
                        pp.tile([128, H // 2, VW], dt.float32, tag="pv0",
                                bufs=1, name="ps_pv0"),
                        pp.tile([128, H // 2, VW], dt.float32, tag="pv1",
                                bufs=1, name="ps_pv1"),
                    )
                for half, h in ((0, 2 * p), (768, 2 * p + 1)):
                    pv_all = pv_tiles[t][h // (H // 2)]
                    hs = h % (H // 2)
                    n_ord = len(PV_ORDER)
                    for i, ci in enumerate(PV_ORDER):
                        if ci < 0:
                            nc.tensor.matmul(
                                pv_all[:, hs, :], et[:, half + 640:half + 768],
                                vglob[:, h * VW:(h + 1) * VW],
                                start=(i == 0), stop=(i == n_ord - 1))
                        else:
                            src = PERM[ci]
                            nc.tensor.matmul(
                                pv_all[:, hs, :],
                                et[:, half + ci * 128:half + (ci + 1) * 128],
                                vext[:, t + src, h * VW:(h + 1) * VW],
                                start=(i == 0), stop=(i == n_ord - 1))

            def emit_normalize_pair(t, p):
                # per-pair normalize, all on vector: keeps scalar exp-only
                # (avoids queue saturation) and frees the single-buffered pv
                # tiles incrementally so the next tile's PV never stalls
                pv_all = pv_tiles[t][(2 * p) // (H // 2)]
                hs0 = (2 * p) % (H // 2)
                rec = mp.tile([128, 2, 1], dt.float32, tag="rec", bufs=4,
                              name="rec")
                nc.vector.reciprocal(rec[:, :, :],
                                     pv_all[:, hs0:hs0 + 2, HD:HD + 1])
                if t not in attn_tiles:
                    attn_tiles[t] = mp.tile([128, H, HD], dt.float32,
                                            tag="attn", bufs=2, name="attn_sb")
                attn_sb = attn_tiles[t]
                for j in range(2):
                    nc.vector.tensor_scalar_mul(attn_sb[:, 2 * p + j, :],
                                                pv_all[:, hs0 + j, 0:HD],
                                                rec[:, j:j + 1, 0:1])
                if p == H // 2 - 1:
                    pv_tiles.pop(t)
                    attn_tiles.pop(t)
                    nc.sync.dma_start(out_band[t * 128:(t + 1) * 128, :],
                                      attn_sb[:, :, :])

            jobs = [(t, p) for t in range(NQT) for p in range(H // 2)]
            pending = None
            for t, p in jobs:
                et = emit_scores(t, p)
                if pending is not None:
                    pt, pp_, pet = pending
                    emit_pv(pt, pp_, pet)
                    emit_normalize_pair(pt, pp_)
                pending = (t, p, et)
            pt, pp_, pet = pending
            emit_pv(pt, pp_, pet)
            emit_normalize_pair(pt, pp_)

            # ---- phase 3: global-token partials over this core's keys ------
            # Per head: all 8 key-chunk score matmuls land in one [128, 512]
            # PSUM tile -> one Exp -> 8 PV accumulation matmuls.  Scores of
            # head h+1 are emitted before the PV of head h.
            gp_sb = mp.tile([G, H, VW], dt.float32)

            def emit_g_scores(h):
                hp, ho = h // 2, (h % 2) * 64
                sc3 = pp.tile([128, 512], dt.float32, tag="sc", bufs=2,
                              name="ps_sc3")
                for c in range(NGC):
                    nc.tensor.matmul(
                        sc3[:, c * 64:(c + 1) * 64],
                        kgT[ho:ho + 64, hp, c * 128:(c + 1) * 128],
                        qgT[ho:ho + 64, hp, :], start=True, stop=True)
                eg = mp.tile([128, 512], dt.bfloat16, tag="eg", bufs=3,
                             name="eg")
                nc.scalar.activation(eg[:, :], sc3[:, :], Act.Exp)
                return eg

            def emit_g_pv(h, eg):
                pg = pp.tile([G, VW], dt.float32, tag="pv0", bufs=1, name="ps_pg")
                for c in range(NGC):
                    nc.tensor.matmul(pg[:, :], eg[:, c * 64:(c + 1) * 64],
                                     vgext[:, c, h * VW:(h + 1) * VW],
                                     start=(c == 0), stop=(c == NGC - 1))
                nc.vector.tensor_copy(gp_sb[:, h, :], pg[:, :])

            g_pending = None
            for h in range(H):
                eg = emit_g_scores(h)
                if g_pending is not None:
                    emit_g_pv(*g_pending)
                g_pending = (h, eg)
            emit_g_pv(*g_pending)
            nc.sync.dma_start(gpart[:, :, :], gp_sb[:, :, :])

    nc.compile()
    return nc


def _get_program():
    global _PROGRAM
    if _PROGRAM is None:
        _PROGRAM = _build_program()
    return _PROGRAM


def kernel(hidden_states, Wq, bq, Wk, bk, Wv, bv, Wqg, bqg, Wkg, bkg, Wvg, bvg,
           attention_mask, n_global):
    from concourse.bass_utils import run_bass_kernel_spmd

    x = np.asarray(hidden_states, np.float32)
    am = np.asarray(attention_mask)
    ng = int(n_global)
    assert ng == G, f"kernel specialized for n_global={G}, got {ng}"
    scale = np.float32(1.0 / np.sqrt(HD))

    def chunked(a):
        # [E, N] -> [128, KE, N] matching the on-chip tile layout
        return np.ascontiguousarray(
            np.asarray(a).reshape(KE, 128, -1).transpose(1, 0, 2))

    wT = {
        "wqt": chunked((np.asarray(Wq, np.float32).T * scale).astype(BF16)),
        "wkt": chunked(np.asarray(Wk, np.float32).T.astype(BF16)),
        "wvt": chunked(np.asarray(Wv, np.float32).T.astype(BF16)),
        "wkgt": chunked(np.asarray(Wkg, np.float32).T.astype(BF16)),
        "wvgt": chunked(np.asarray(Wvg, np.float32).T.astype(BF16)),
        "wqgt": chunked((np.asarray(Wqg, np.float32).T * scale).astype(BF16)),
    }
    for bias in (bq, bk, bv, bqg, bkg, bvg):
        assert not np.any(np.asarray(bias)), "nonzero biases unsupported"

    tril = np.tril(np.ones((128, 128), np.float32))
    triu = np.triu(np.ones((128, 128), np.float32))
    mask2 = np.concatenate([tril, triu], axis=1).astype(BF16)

    xTg_pad = np.zeros((E, GP), np.float32)

    in_maps = []
    for core in range(NCORES):
        b, cb = divmod(core, 4)
        r0 = cb * CHUNK
        lo, hi = r0 - W, r0 + CHUNK + W
        slab = np.zeros((SLAB, E), np.float32)
        s0, s1 = max(0, lo), min(S, hi)
        slab[s0 - lo: s1 - lo] = x[b, s0:s1]
        valid = np.zeros(SLAB, np.float32)
        arange = np.arange(lo, hi)
        inb = (arange >= 0) & (arange < S)
        valid[inb] = (am[b, arange[inb]] == 0).astype(np.float32)
        gvalid = np.ones(G, np.float32)  # keys < n_global (ng == G)

        xtg = xTg_pad.copy()
        xtg[:, :G] = x[b, :G].T

        in_maps.append({
            "xT": chunked(np.ascontiguousarray(slab.T).astype(BF16)),
            "xTg": chunked(xtg.astype(BF16)),
            **wT,
            "vmask": np.ascontiguousarray(valid.reshape(NSC, 128).T),
            "vones": np.ascontiguousarray(
                np.repeat(valid.reshape(NSC, 128).T[:, :, None], H, axis=2)
            ).astype(BF16),
            "gmask": gvalid[:, None].copy(),
            "gones": np.repeat(gvalid[:, None], H, axis=1).astype(BF16),
            "mask2": mask2,
        })

    nc = _get_program()
    globals()['_last_in_maps'] = in_maps
    res = run_bass_kernel_spmd(nc, in_maps, core_ids=list(range(NCORES)))

    out = np.empty((B, S, E), np.float32)
    for core in range(NCORES):
        b, cb = divmod(core, 4)
        out[b, cb * CHUNK:(cb + 1) * CHUNK] = res.results[core]["out_band"]
    for b in range(B):
        acc = sum(res.results[b * 4 + cb]["gpart"] for cb in range(4))
        gout = acc[:, :, 0:HD] / acc[:, :, HD:HD + 1]
        out[b, :G] = gout.reshape(G, E)
    return out


# revision 19
# speedup vs baseline: 1.0810x; 1.0810x over previous
"""Longformer self-attention Bass kernel for 8 Trainium2 NeuronCores.

Sharding: data-parallel over batch (2) x sequence-parallel (4 chunks of 1024
queries). Each core receives a transposed x slab covering its rows +-W halo
(zero padded at sequence edges), the 64 global rows, and the full weight set.

On-device layout choices:
  - "T layout" [E_out on partitions, seq on free] for q/k/kg/qg so the banded
    score matmuls need no transposes; head pairs share 128-partition slices.
  - Scores are computed keys-on-partitions (S^T). Softmax runs without max
    subtraction (scores are O(1)); the denominator comes for free from a
    ones-column appended to v, so P@V and sum(exp) are one PSUM accumulation.
  - The global keys are zero-padded to 128 so their scores live in the same
    PSUM tile as the band ([128, 768] per head: 5 band chunks + glob block)
    and one Exp covers everything; padded keys get exp(0)=1 but multiply
    zeroed v/ones columns, contributing nothing.
  - Band-edge masking: score chunk order is [c0, c4, c1, c2, c3] so the two
    masked chunks are adjacent and one [128,256] mask multiply per head
    handles the window geometry.
  - Software pipelining: scores for head-pair p+1 are emitted before the PV
    matmuls of pair p, hiding the exp/mask latency behind PE work.
  - Per query tile, all 12 heads' PV accumulate into one PSUM tile
    [128, H, 65]; one strided reciprocal per tile replaces 12, and the
    normalize scale-muls run on the otherwise idle GpSimd engine.
  - All masking is data-driven (uniform SPMD program): key validity (padding
    + global-key exclusion from the band) is folded into the v/ones columns.
"""

import sys
import numpy as np

if '/opt/trn_rl_repo' not in sys.path:
    sys.path.insert(0, '/opt/trn_rl_repo')

import ml_dtypes

B, S, E, H, HD, G, W = 2, 4096, 768, 12, 64, 64, 256
NCORES = 8
CHUNK = 1024          # query rows per core
SLAB = 1536           # halo slab rows per core (CHUNK + 2W)
NQT = CHUNK // 128    # query tiles per core
NSC = SLAB // 128     # slab chunks of 128 keys
NGC = CHUNK // 128    # key chunks for the global-token partials
KE = E // 128         # contraction chunks over E
VW = HD + 1           # v head width incl. ones column
GP = 128              # global keys padded to 128
BF16 = ml_dtypes.bfloat16

# score-tile column order: key chunks [0, 4, 1, 2, 3]; masked chunks first.
PERM = (0, 4, 1, 2, 3)
# PV consumption order: unmasked middle chunks, then glob, then masked edges
PV_ORDER = (2, 3, 4, -1, 0, 1)   # -1 = glob block (cols 640:768)

_PROGRAM = None


def _build_program():
    import concourse.bass as bass
    import concourse.mybir as mybir
    import concourse.tile as tile
    from concourse import bacc

    dt = mybir.dt
    Act = mybir.ActivationFunctionType

    nc = bacc.Bacc("TRN2", target_bir_lowering=False, debug=False,
                   num_devices=NCORES)

    xT = nc.dram_tensor("xT", [128, KE, SLAB], dt.bfloat16, kind="ExternalInput")
    xTg = nc.dram_tensor("xTg", [128, KE, GP], dt.bfloat16, kind="ExternalInput")
    wts = {}
    for nm in ("wqt", "wkt", "wvt", "wkgt", "wvgt", "wqgt"):
        wts[nm] = nc.dram_tensor(nm, [128, KE, E], dt.bfloat16, kind="ExternalInput")
    vmask_d = nc.dram_tensor("vmask", [128, NSC], dt.float32, kind="ExternalInput")
    vones_d = nc.dram_tensor("vones", [128, NSC, H], dt.bfloat16, kind="ExternalInput")
    gmask_d = nc.dram_tensor("gmask", [G, 1], dt.float32, kind="ExternalInput")
    gones_d = nc.dram_tensor("gones", [G, H], dt.bfloat16, kind="ExternalInput")
    mask2_d = nc.dram_tensor("mask2", [128, 256], dt.bfloat16, kind="ExternalInput")
    out_band = nc.dram_tensor("out_band", [CHUNK, E], dt.float32, kind="ExternalOutput")
    gpart = nc.dram_tensor("gpart", [G, H, VW], dt.float32, kind="ExternalOutput")

    with tile.TileContext(nc) as tc:
        with tc.tile_pool(name="main", bufs=1) as mp, \
             tc.tile_pool(name="psum", bufs=1, space="PSUM") as pp:

            # ---- resident SBUF tensors -------------------------------------
            xT_sb = mp.tile([128, KE, SLAB], dt.bfloat16)
            xTg_sb = mp.tile([128, KE, GP], dt.bfloat16)
            w_sb = {nm: mp.tile([128, KE, E], dt.bfloat16, name=f"w_{nm}")
                    for nm in wts}
            qTzA = mp.tile([128, KE, CHUNK], dt.bfloat16, name="qTzA")
            qTzB = mp.tile([128, KE, CHUNK], dt.bfloat16, name="qTzB")
            kT = mp.tile([128, KE, SLAB], dt.bfloat16)
            kgT = mp.tile([128, KE, CHUNK], dt.bfloat16)
            kglobT = mp.tile([128, KE, GP], dt.bfloat16)
            qgT = mp.tile([128, KE, G], dt.bfloat16)
            vext = mp.tile([128, NSC, H * VW], dt.bfloat16)
            vgext = mp.tile([128, NGC, H * VW], dt.bfloat16)
            vglob = mp.tile([GP, H * VW], dt.bfloat16)
            vmask = mp.tile([128, NSC], dt.float32)
            gmask = mp.tile([G, 1], dt.float32)
            mask2 = mp.tile([128, 256], dt.bfloat16)

            nc.sync.dma_start(xT_sb[:, :, :], xT[:, :, :])
            nc.sync.dma_start(xTg_sb[:, :, :], xTg[:, :, :])
            for nm in wts:
                nc.sync.dma_start(w_sb[nm][:, :, :], wts[nm][:, :, :])
            nc.sync.dma_start(vmask[:, :], vmask_d[:, :])
            nc.sync.dma_start(gmask[:, :], gmask_d[:, :])
            nc.sync.dma_start(mask2[:, :], mask2_d[:, :])
            # vglob rows G:GP stay zero (padded global keys kill their own
            # exp(0)=1 contributions); zero everything, then fill rows 0:G.
            nc.vector.memset(vglob[:, :], 0.0)
            nc.gpsimd.memset(qTzA[64:128, :, :], 0.0)
            nc.gpsimd.memset(qTzB[0:64, :, :], 0.0)
            # ones columns (gated by key validity)
            nc.sync.dma_start(
                vext.rearrange("p c (h e) -> p c h e", e=VW)[:, :, :, HD:HD + 1],
                vones_d.rearrange("p c (h o) -> p c h o", o=1))
            nc.sync.dma_start(
                vglob.rearrange("p (h e) -> p h e", e=VW)[0:G, :, HD:HD + 1],
                gones_d.rearrange("p (h o) -> p h o", o=1))
            nc.vector.memset(
                vgext.rearrange("p c (h e) -> p c h e", e=VW)[:, :, :, HD:HD + 1], 1.0)

            # ---- phase 1: projections --------------------------------------
            def proj_T(dst, wname, src, src_off, n_total, bias=None):
                # dst[e_out, n] = sum_e w[e, e_out] * src[e, src_off + n]
                for m in range(KE):
                    n0 = 0
                    while n0 < n_total:
                        nw = min(512, n_total - n0)
                        ps = pp.tile([128, 512], dt.float32, tag="sc", bufs=3,
                                     name="ps_projT")
                        for k in range(KE):
                            nc.tensor.matmul(
                                ps[:, :nw],
                                w_sb[wname][:, k, m * 128:(m + 1) * 128],
                                src[:, k, src_off + n0: src_off + n0 + nw],
                                start=(k == 0), stop=(k == KE - 1))
                        nc.scalar.activation(dst[:, m, n0:n0 + nw], ps[:, :nw],
                                             Act.Copy)
                        n0 += nw

            def proj_q():
                # like proj_T but splits each 128-row block between the two
                # zero-padded q tensors (head 2m rows 0:64 -> qTzA, head 2m+1
                # rows 64:128 -> qTzB) so score matmuls of a head pair can
                # share one full-128-row kT stationary
                for m in range(KE):
                    n0 = 0
                    while n0 < CHUNK:
                        nw = min(512, CHUNK - n0)
                        ps = pp.tile([128, 512], dt.float32, tag="sc", bufs=3,
                                     name="ps_projq")
                        for k in range(KE):
                            nc.tensor.matmul(
                                ps[:, :nw],
                                w_sb["wqt"][:, k, m * 128:(m + 1) * 128],
                                xT_sb[:, k, W + n0: W + n0 + nw],
                                start=(k == 0), stop=(k == KE - 1))
                        nc.scalar.activation(qTzA[0:64, m, n0:n0 + nw],
                                             ps[0:64, :nw], Act.Copy)
                        nc.scalar.activation(qTzB[64:128, m, n0:n0 + nw],
                                             ps[64:128, :nw], Act.Copy)
                        n0 += nw

            proj_q()
            proj_T(kT, "wkt", xT_sb, 0, SLAB)
            proj_T(kglobT, "wkt", xTg_sb, 0, GP)
            proj_T(kgT, "wkgt", xT_sb, W, CHUNK)
            proj_T(qgT, "wqgt", xTg_sb, 0, G)

            def proj_V(dst, wname, src, src_off, n_chunks, mask_tile):
                # dst[seq, h, d] = sum_e src[e, seq] * w[e, 64h+d], * mask[seq]
                for sI in range(n_chunks):
                    for h0, nw in ((0, 512), (512, 256)):
                        ps = pp.tile([128, 512], dt.float32, tag="sc", bufs=3,
                                     name="ps_projV")
                        for k in range(KE):
                            nc.tensor.matmul(
                                ps[:, :nw],
                                src[:, k, src_off + sI * 128: src_off + (sI + 1) * 128],
                                w_sb[wname][:, k, h0:h0 + nw],
                                start=(k == 0), stop=(k == KE - 1))
                        dv = dst.rearrange("p c (h e) -> p c h e", e=VW)[
                            :, sI, h0 // HD:(h0 + nw) // HD, 0:HD]
                        pv = ps[:, :nw].rearrange("p (h e) -> p h e", e=HD)
                        if mask_tile is None:
                            nc.vector.tensor_copy(dv, pv)
                        else:
                            nc.vector.tensor_scalar_mul(dv, pv,
                                                        mask_tile[:, sI:sI + 1])

            # vext v-data is gated by per-key validity (padding + global-key
            # exclusion from the band); vgext is not masked (global-token
            # attention sees every key).
            proj_V(vext, "wvt", xT_sb, 0, NSC, vmask)
            proj_V(vgext, "wvgt", xT_sb, W, NGC, None)

            # v of the global rows (for the global-key part of the band
            # softmax), gated by j < n_global
            for h0, nw in ((0, 512), (512, 256)):
                psg = pp.tile([128, 512], dt.float32, tag="sc", bufs=3,
                              name="ps_vglob")
                for k in range(KE):
                    nc.tensor.matmul(
                        psg[0:G, :nw], xTg_sb[:, k, 0:G], w_sb["wvt"][:, k, h0:h0 + nw],
                        start=(k == 0), stop=(k == KE - 1))
                dv = vglob.rearrange("p (h e) -> p h e", e=VW)[
                    0:G, h0 // HD:(h0 + nw) // HD, 0:HD]
                nc.vector.tensor_scalar_mul(
                    dv, psg[0:G, :nw].rearrange("p (h e) -> p h e", e=HD),
                    gmask[:, :])

            # ---- phase 2: banded + global-key attention --------------------
            # Per head: one [128, 768] PSUM score tile (5 band chunks in PERM
            # order + padded glob block), one Exp, one [128,256] edge mask.
            # Scores of pair p+1 are emitted before the PV of pair p so the
            # exp/mask chain hides behind PE work.
            pv_tiles = {}      # t -> psum accumulator [128, H, VW]
            attn_tiles = {}

            def emit_scores(t, p):
                hp = p
                qsA = qTzA[:, hp, t * 128:(t + 1) * 128]
                qsB = qTzB[:, hp, t * 128:(t + 1) * 128]
                scA = pp.tile([128, 768], dt.float32, tag="sc", bufs=3,
                              name="ps_scA")
                scB = pp.tile([128, 768], dt.float32, tag="sc", bufs=3,
                              name="ps_scB")
                # full-128-row stationary (both heads' k dims); the moving q
                # is zero-padded on the other head's rows, so each head's
                # scores are exact and consecutive A/B matmuls reuse the
                # loaded weights
                for ci, src in enumerate(PERM):
                    kst = kT[:, hp, (t + src) * 128:(t + src + 1) * 128]
                    nc.tensor.matmul(scA[:, ci * 128:(ci + 1) * 128], kst,
                                     qsA, start=True, stop=True)
                    nc.tensor.matmul(scB[:, ci * 128:(ci + 1) * 128], kst,
                                     qsB, start=True, stop=True)
                nc.tensor.matmul(scA[:, 640:768], kglobT[:, hp, :],
                                 qsA, start=True, stop=True)
                nc.tensor.matmul(scB[:, 640:768], kglobT[:, hp, :],
                                 qsB, start=True, stop=True)
                ets = []
                for sc, nm in ((scA, "A"), (scB, "B")):
                    et = mp.tile([128, 768], dt.bfloat16, tag="et", bufs=6,
                                 name=f"et{nm}")
                    nc.scalar.activation(et[:, :], sc[:, :], Act.Exp)
                    nc.vector.tensor_mul(et[:, 0:256], et[:, 0:256], mask2[:, :])
                    ets.append(et)
                return ets

            def emit_pv(t, p, etA, etB):
                # two 6-head accumulators: 6*65 floats = 390 <= 512 keeps every
                # per-head matmul output inside one PSUM bank
                if t not in pv_tiles:
                    pv_tiles[t] = (
                        pp.tile([128, H // 2, VW], dt.float32, tag="pv0",
                                bufs=1, name="ps_pv0"),
                        pp.tile([128, H // 2, VW], dt.float32, tag="pv1",
                                bufs=1, name="ps_pv1"),
                    )
                for et, h in ((etA, 2 * p), (etB, 2 * p + 1)):
                    pv_all = pv_tiles[t][h // (H // 2)]
                    hs = h % (H // 2)
                    n_ord = len(PV_ORDER)
                    for i, ci in enumerate(PV_ORDER):
                        if ci < 0:
                            nc.tensor.matmul(
                                pv_all[:, hs, :], et[:, 640:768],
                                vglob[:, h * VW:(h + 1) * VW],
                                start=(i == 0), stop=(i == n_ord - 1))
                        else:
                            src = PERM[ci]
                            nc.tensor.matmul(
                                pv_all[:, hs, :],
                                et[:, ci * 128:(ci + 1) * 128],
                                vext[:, t + src, h * VW:(h + 1) * VW],
                                start=(i == 0), stop=(i == n_ord - 1))

            def emit_normalize_pair(t, p):
                # per-pair normalize, all on vector: keeps scalar exp-only
                # (avoids queue saturation) and frees the single-buffered pv
                # tiles incrementally so the next tile's PV never stalls
                pv_all = pv_tiles[t][(2 * p) // (H // 2)]
                hs0 = (2 * p) % (H // 2)
                rec = mp.tile([128, 2, 1], dt.float32, tag="rec", bufs=4,
                              name="rec")
                nc.vector.reciprocal(rec[:, :, :],
                                     pv_all[:, hs0:hs0 + 2, HD:HD + 1])
                if t not in attn_tiles:
                    attn_tiles[t] = mp.tile([128, H, HD], dt.float32,
                                            tag="attn", bufs=2, name="attn_sb")
                attn_sb = attn_tiles[t]
                for j in range(2):
                    nc.vector.tensor_scalar_mul(attn_sb[:, 2 * p + j, :],
                                                pv_all[:, hs0 + j, 0:HD],
                                                rec[:, j:j + 1, 0:1])
                if p == H // 2 - 1:
                    pv_tiles.pop(t)
                    attn_tiles.pop(t)
                    nc.sync.dma_start(out_band[t * 128:(t + 1) * 128, :],
                                      attn_sb[:, :, :])

            jobs = [(t, p) for t in range(NQT) for p in range(H // 2)]
            pending = None
            for t, p in jobs:
                ets = emit_scores(t, p)
                if pending is not None:
                    pt, pp_, petA, petB = pending
                    emit_pv(pt, pp_, petA, petB)
                    emit_normalize_pair(pt, pp_)
                pending = (t, p, ets[0], ets[1])
            pt, pp_, petA, petB = pending
            emit_pv(pt, pp_, petA, petB)
            emit_normalize_pair(pt, pp_)

            # ---- phase 3: global-token partials over this core's keys ------
            # Per head: all 8 key-chunk score matmuls land in one [128, 512]
            # PSUM tile -> one Exp -> 8 PV accumulation matmuls.  Scores of
            # head h+1 are emitted before the PV of head h.
            gp_sb = mp.tile([G, H, VW], dt.float32)

            def emit_g_scores(h):
                hp, ho = h // 2, (h % 2) * 64
                sc3 = pp.tile([128, 512], dt.float32, tag="sc", bufs=3,
                              name="ps_sc3")
                for c in range(NGC):
                    nc.tensor.matmul(
                        sc3[:, c * 64:(c + 1) * 64],
                        kgT[ho:ho + 64, hp, c * 128:(c + 1) * 128],
                        qgT[ho:ho + 64, hp, :], start=True, stop=True)
                eg = mp.tile([128, 512], dt.bfloat16, tag="eg", bufs=3,
                             name="eg")
                nc.scalar.activation(eg[:, :], sc3[:, :], Act.Exp)
                return eg

            def emit_g_pv(h, eg):
                pg = pp.tile([G, VW], dt.float32, tag="pv0", bufs=1, name="ps_pg")
                for c in range(NGC):
                    nc.tensor.matmul(pg[:, :], eg[:, c * 64:(c + 1) * 64],
                                     vgext[:, c, h * VW:(h + 1) * VW],
                                     start=(c == 0), stop=(c == NGC - 1))
                nc.vector.tensor_copy(gp_sb[:, h, :], pg[:, :])

            g_pending = None
            for h in range(H):
                eg = emit_g_scores(h)
                if g_pending is not None:
                    emit_g_pv(*g_pending)
                g_pending = (h, eg)
            emit_g_pv(*g_pending)
            nc.sync.dma_start(gpart[:, :, :], gp_sb[:, :, :])

    nc.compile()
    return nc


def _get_program():
    global _PROGRAM
    if _PROGRAM is None:
        _PROGRAM = _build_program()
    return _PROGRAM


def kernel(hidden_states, Wq, bq, Wk, bk, Wv, bv, Wqg, bqg, Wkg, bkg, Wvg, bvg,
           attention_mask, n_global):
    from concourse.bass_utils import run_bass_kernel_spmd

    x = np.asarray(hidden_states, np.float32)
    am = np.asarray(attention_mask)
    ng = int(n_global)
    assert ng == G, f"kernel specialized for n_global={G}, got {ng}"
    scale = np.float32(1.0 / np.sqrt(HD))

    def chunked(a):
        # [E, N] -> [128, KE, N] matching the on-chip tile layout
        return np.ascontiguousarray(
            np.asarray(a).reshape(KE, 128, -1).transpose(1, 0, 2))

    wT = {
        "wqt": chunked((np.asarray(Wq, np.float32).T * scale).astype(BF16)),
        "wkt": chunked(np.asarray(Wk, np.float32).T.astype(BF16)),
        "wvt": chunked(np.asarray(Wv, np.float32).T.astype(BF16)),
        "wkgt": chunked(np.asarray(Wkg, np.float32).T.astype(BF16)),
        "wvgt": chunked(np.asarray(Wvg, np.float32).T.astype(BF16)),
        "wqgt": chunked((np.asarray(Wqg, np.float32).T * scale).astype(BF16)),
    }
    for bias in (bq, bk, bv, bqg, bkg, bvg):
        assert not np.any(np.asarray(bias)), "nonzero biases unsupported"

    tril = np.tril(np.ones((128, 128), np.float32))
    triu = np.triu(np.ones((128, 128), np.float32))
    mask2 = np.concatenate([tril, triu], axis=1).astype(BF16)

    xTg_pad = np.zeros((E, GP), np.float32)

    in_maps = []
    for core in range(NCORES):
        b, cb = divmod(core, 4)
        r0 = cb * CHUNK
        lo, hi = r0 - W, r0 + CHUNK + W
        slab = np.zeros((SLAB, E), np.float32)
        s0, s1 = max(0, lo), min(S, hi)
        slab[s0 - lo: s1 - lo] = x[b, s0:s1]
        valid = np.zeros(SLAB, np.float32)
        arange = np.arange(lo, hi)
        inb = (arange >= 0) & (arange < S)
        valid[inb] = (am[b, arange[inb]] == 0).astype(np.float32)
        gvalid = np.ones(G, np.float32)  # keys < n_global (ng == G)

        xtg = xTg_pad.copy()
        xtg[:, :G] = x[b, :G].T

        in_maps.append({
            "xT": chunked(np.ascontiguousarray(slab.T).astype(BF16)),
            "xTg": chunked(xtg.astype(BF16)),
            **wT,
            "vmask": np.ascontiguousarray(valid.reshape(NSC, 128).T),
            "vones": np.ascontiguousarray(
                np.repeat(valid.reshape(NSC, 128).T[:, :, None], H, axis=2)
            ).astype(BF16),
            "gmask": gvalid[:, None].copy(),
            "gones": np.repeat(gvalid[:, None], H, axis=1).astype(BF16),
            "mask2": mask2,
        })

    nc = _get_program()
    globals()['_last_in_maps'] = in_maps
    res = run_bass_kernel_spmd(nc, in_maps, core_ids=list(range(NCORES)))

    out = np.empty((B, S, E), np.float32)
    for core in range(NCORES):
        b, cb = divmod(core, 4)
        out[b, cb * CHUNK:(cb + 1) * CHUNK] = res.results[core]["out_band"]
    for b in range(B):
        acc = sum(res.results[b * 4 + cb]["gpart"] for cb in range(4))
        gout = acc[:, :, 0:HD] / acc[:, :, HD:HD + 1]
        out[b, :G] = gout.reshape(G, E)
    return out


# revision 20
# speedup vs baseline: 1.0885x; 1.0069x over previous
"""Longformer self-attention Bass kernel for 8 Trainium2 NeuronCores.

Sharding: data-parallel over batch (2) x sequence-parallel (4 chunks of 1024
queries). Each core receives a transposed x slab covering its rows +-W halo
(zero padded at sequence edges), the 64 global rows, and the full weight set.

On-device layout choices:
  - "T layout" [E_out on partitions, seq on free] for q/k/kg/qg so the banded
    score matmuls need no transposes; head pairs share 128-partition slices.
  - Scores are computed keys-on-partitions (S^T). Softmax runs without max
    subtraction (scores are O(1)); the denominator comes for free from a
    ones-column appended to v, so P@V and sum(exp) are one PSUM accumulation.
  - The global keys are zero-padded to 128 so their scores live in the same
    PSUM tile as the band ([128, 768] per head: 5 band chunks + glob block)
    and one Exp covers everything; padded keys get exp(0)=1 but multiply
    zeroed v/ones columns, contributing nothing.
  - Band-edge masking: score chunk order is [c0, c4, c1, c2, c3] so the two
    masked chunks are adjacent and one [128,256] mask multiply per head
    handles the window geometry.
  - Software pipelining: scores for head-pair p+1 are emitted before the PV
    matmuls of pair p, hiding the exp/mask latency behind PE work.
  - Per query tile, all 12 heads' PV accumulate into one PSUM tile
    [128, H, 65]; one strided reciprocal per tile replaces 12, and the
    normalize scale-muls run on the otherwise idle GpSimd engine.
  - All masking is data-driven (uniform SPMD program): key validity (padding
    + global-key exclusion from the band) is folded into the v/ones columns.
"""

import sys
import numpy as np

if '/opt/trn_rl_repo' not in sys.path:
    sys.path.insert(0, '/opt/trn_rl_repo')

import ml_dtypes

B, S, E, H, HD, G, W = 2, 4096, 768, 12, 64, 64, 256
NCORES = 8
CHUNK = 1024          # query rows per core
SLAB = 1536           # halo slab rows per core (CHUNK + 2W)
NQT = CHUNK // 128    # query tiles per core
NSC = SLAB // 128     # slab chunks of 128 keys
NGC = CHUNK // 128    # key chunks for the global-token partials
KE = E // 128         # contraction chunks over E
VW = HD + 1           # v head width incl. ones column
GP = 128              # global keys padded to 128
BF16 = ml_dtypes.bfloat16

# score-tile column order: key chunks [0, 4, 1, 2, 3]; masked chunks first.
PERM = (0, 4, 1, 2, 3)
# PV consumption order: unmasked middle chunks, then glob, then masked edges
PV_ORDER = (2, 3, 4, -1, 0, 1)   # -1 = glob block (cols 640:768)

_PROGRAM = None


def _build_program():
    import concourse.bass as bass
    import concourse.mybir as mybir
    import concourse.tile as tile
    from concourse import bacc

    dt = mybir.dt
    Act = mybir.ActivationFunctionType

    nc = bacc.Bacc("TRN2", target_bir_lowering=False, debug=False,
                   num_devices=NCORES)

    xT = nc.dram_tensor("xT", [128, KE, SLAB], dt.bfloat16, kind="ExternalInput")
    xTg = nc.dram_tensor("xTg", [128, KE, GP], dt.bfloat16, kind="ExternalInput")
    wts = {}
    for nm in ("wqt", "wkt", "wvt", "wkgt", "wvgt", "wqgt"):
        wts[nm] = nc.dram_tensor(nm, [128, KE, E], dt.bfloat16, kind="ExternalInput")
    vmask_d = nc.dram_tensor("vmask", [128, NSC], dt.float32, kind="ExternalInput")
    vones_d = nc.dram_tensor("vones", [128, NSC, H], dt.bfloat16, kind="ExternalInput")
    gmask_d = nc.dram_tensor("gmask", [G, 1], dt.float32, kind="ExternalInput")
    gones_d = nc.dram_tensor("gones", [G, H], dt.bfloat16, kind="ExternalInput")
    mask2_d = nc.dram_tensor("mask2", [128, 256], dt.bfloat16, kind="ExternalInput")
    out_band = nc.dram_tensor("out_band", [CHUNK, E], dt.float32, kind="ExternalOutput")
    gpart = nc.dram_tensor("gpart", [G, H, VW], dt.float32, kind="ExternalOutput")

    with tile.TileContext(nc) as tc:
        with tc.tile_pool(name="main", bufs=1) as mp, \
             tc.tile_pool(name="psum", bufs=1, space="PSUM") as pp:

            # ---- resident SBUF tensors -------------------------------------
            xT_sb = mp.tile([128, KE, SLAB], dt.bfloat16)
            xTg_sb = mp.tile([128, KE, GP], dt.bfloat16)
            w_sb = {nm: mp.tile([128, KE, E], dt.bfloat16, name=f"w_{nm}")
                    for nm in wts}
            qTzA = mp.tile([128, KE, CHUNK], dt.bfloat16, name="qTzA")
            qTzB = mp.tile([128, KE, CHUNK], dt.bfloat16, name="qTzB")
            kT = mp.tile([128, KE, SLAB], dt.bfloat16)
            kgT = mp.tile([128, KE, CHUNK], dt.bfloat16)
            kglobT = mp.tile([128, KE, GP], dt.bfloat16)
            qgTzA = mp.tile([128, KE, G], dt.bfloat16, name="qgTzA")
            qgTzB = mp.tile([128, KE, G], dt.bfloat16, name="qgTzB")
            vext = mp.tile([128, NSC, H * VW], dt.bfloat16)
            vgext = mp.tile([128, NGC, H * VW], dt.bfloat16)
            vglob = mp.tile([GP, H * VW], dt.bfloat16)
            vmask = mp.tile([128, NSC], dt.float32)
            gmask = mp.tile([G, 1], dt.float32)
            mask2 = mp.tile([128, 256], dt.bfloat16)

            nc.sync.dma_start(xT_sb[:, :, :], xT[:, :, :])
            nc.sync.dma_start(xTg_sb[:, :, :], xTg[:, :, :])
            for nm in ("wqt", "wkt"):
                nc.sync.dma_start(w_sb[nm][:, :, :], wts[nm][:, :, :])
            nc.sync.dma_start(vmask[:, :], vmask_d[:, :])
            nc.sync.dma_start(gmask[:, :], gmask_d[:, :])
            nc.sync.dma_start(mask2[:, :], mask2_d[:, :])
            # vglob rows G:GP stay zero (padded global keys kill their own
            # exp(0)=1 contributions); zero everything, then fill rows 0:G.
            nc.vector.memset(vglob[:, :], 0.0)
            nc.gpsimd.memset(qTzA[64:128, :, :], 0.0)
            nc.gpsimd.memset(qTzB[0:64, :, :], 0.0)
            nc.gpsimd.memset(qgTzA[64:128, :, :], 0.0)
            nc.gpsimd.memset(qgTzB[0:64, :, :], 0.0)
            # ones columns (gated by key validity)
            nc.sync.dma_start(
                vext.rearrange("p c (h e) -> p c h e", e=VW)[:, :, :, HD:HD + 1],
                vones_d.rearrange("p c (h o) -> p c h o", o=1))
            nc.sync.dma_start(
                vglob.rearrange("p (h e) -> p h e", e=VW)[0:G, :, HD:HD + 1],
                gones_d.rearrange("p (h o) -> p h o", o=1))
            nc.vector.memset(
                vgext.rearrange("p c (h e) -> p c h e", e=VW)[:, :, :, HD:HD + 1], 1.0)

            # ---- phase 1: projections --------------------------------------
            def proj_T(dst, wname, src, src_off, n_total, bias=None):
                # dst[e_out, n] = sum_e w[e, e_out] * src[e, src_off + n]
                for m in range(KE):
                    n0 = 0
                    while n0 < n_total:
                        nw = min(512, n_total - n0)
                        ps = pp.tile([128, 512], dt.float32, tag="sc", bufs=3,
                                     name="ps_projT")
                        for k in range(KE):
                            nc.tensor.matmul(
                                ps[:, :nw],
                                w_sb[wname][:, k, m * 128:(m + 1) * 128],
                                src[:, k, src_off + n0: src_off + n0 + nw],
                                start=(k == 0), stop=(k == KE - 1))
                        nc.scalar.activation(dst[:, m, n0:n0 + nw], ps[:, :nw],
                                             Act.Copy)
                        n0 += nw

            def proj_q():
                # like proj_T but splits each 128-row block between the two
                # zero-padded q tensors (head 2m rows 0:64 -> qTzA, head 2m+1
                # rows 64:128 -> qTzB) so score matmuls of a head pair can
                # share one full-128-row kT stationary
                for m in range(KE):
                    n0 = 0
                    while n0 < CHUNK:
                        nw = min(512, CHUNK - n0)
                        ps = pp.tile([128, 512], dt.float32, tag="sc", bufs=3,
                                     name="ps_projq")
                        for k in range(KE):
                            nc.tensor.matmul(
                                ps[:, :nw],
                                w_sb["wqt"][:, k, m * 128:(m + 1) * 128],
                                xT_sb[:, k, W + n0: W + n0 + nw],
                                start=(k == 0), stop=(k == KE - 1))
                        nc.scalar.activation(qTzA[0:64, m, n0:n0 + nw],
                                             ps[0:64, :nw], Act.Copy)
                        nc.scalar.activation(qTzB[64:128, m, n0:n0 + nw],
                                             ps[64:128, :nw], Act.Copy)
                        n0 += nw

            proj_q()
            for nm in ("wvt", "wkgt", "wvgt", "wqgt"):
                nc.sync.dma_start(w_sb[nm][:, :, :], wts[nm][:, :, :])
            proj_T(kT, "wkt", xT_sb, 0, SLAB)
            proj_T(kglobT, "wkt", xTg_sb, 0, GP)
            proj_T(kgT, "wkgt", xT_sb, W, CHUNK)
            for m in range(KE):
                psq = pp.tile([128, 512], dt.float32, tag="sc", bufs=3,
                              name="ps_projqg")
                for k in range(KE):
                    nc.tensor.matmul(
                        psq[:, :G], w_sb["wqgt"][:, k, m * 128:(m + 1) * 128],
                        xTg_sb[:, k, 0:G], start=(k == 0), stop=(k == KE - 1))
                nc.scalar.activation(qgTzA[0:64, m, :], psq[0:64, :G], Act.Copy)
                nc.scalar.activation(qgTzB[64:128, m, :], psq[64:128, :G],
                                     Act.Copy)

            def proj_V(dst, wname, src, src_off, n_chunks, mask_tile):
                # dst[seq, h, d] = sum_e src[e, seq] * w[e, 64h+d], * mask[seq]
                for sI in range(n_chunks):
                    for h0, nw in ((0, 512), (512, 256)):
                        ps = pp.tile([128, 512], dt.float32, tag="sc", bufs=3,
                                     name="ps_projV")
                        for k in range(KE):
                            nc.tensor.matmul(
                                ps[:, :nw],
                                src[:, k, src_off + sI * 128: src_off + (sI + 1) * 128],
                                w_sb[wname][:, k, h0:h0 + nw],
                                start=(k == 0), stop=(k == KE - 1))
                        dv = dst.rearrange("p c (h e) -> p c h e", e=VW)[
                            :, sI, h0 // HD:(h0 + nw) // HD, 0:HD]
                        pv = ps[:, :nw].rearrange("p (h e) -> p h e", e=HD)
                        if mask_tile is None:
                            nc.vector.tensor_copy(dv, pv)
                        else:
                            nc.vector.tensor_scalar_mul(dv, pv,
                                                        mask_tile[:, sI:sI + 1])

            # vext v-data is gated by per-key validity (padding + global-key
            # exclusion from the band); vgext is not masked (global-token
            # attention sees every key).
            proj_V(vext, "wvt", xT_sb, 0, NSC, vmask)
            proj_V(vgext, "wvgt", xT_sb, W, NGC, None)

            # v of the global rows (for the global-key part of the band
            # softmax), gated by j < n_global
            for h0, nw in ((0, 512), (512, 256)):
                psg = pp.tile([128, 512], dt.float32, tag="sc", bufs=3,
                              name="ps_vglob")
                for k in range(KE):
                    nc.tensor.matmul(
                        psg[0:G, :nw], xTg_sb[:, k, 0:G], w_sb["wvt"][:, k, h0:h0 + nw],
                        start=(k == 0), stop=(k == KE - 1))
                dv = vglob.rearrange("p (h e) -> p h e", e=VW)[
                    0:G, h0 // HD:(h0 + nw) // HD, 0:HD]
                nc.vector.tensor_scalar_mul(
                    dv, psg[0:G, :nw].rearrange("p (h e) -> p h e", e=HD),
                    gmask[:, :])

            # ---- phase 2: banded + global-key attention --------------------
            # Per head: one [128, 768] PSUM score tile (5 band chunks in PERM
            # order + padded glob block), one Exp, one [128,256] edge mask.
            # Scores of pair p+1 are emitted before the PV of pair p so the
            # exp/mask chain hides behind PE work.
            pv_tiles = {}      # t -> psum accumulator [128, H, VW]
            attn_tiles = {}

            def emit_scores(t, p):
                hp = p
                qsA = qTzA[:, hp, t * 128:(t + 1) * 128]
                qsB = qTzB[:, hp, t * 128:(t + 1) * 128]
                scA = pp.tile([128, 768], dt.float32, tag="sc", bufs=3,
                              name="ps_scA")
                scB = pp.tile([128, 768], dt.float32, tag="sc", bufs=3,
                              name="ps_scB")
                # full-128-row stationary (both heads' k dims); the moving q
                # is zero-padded on the other head's rows, so each head's
                # scores are exact and consecutive A/B matmuls reuse the
                # loaded weights
                for ci, src in enumerate(PERM):
                    kst = kT[:, hp, (t + src) * 128:(t + src + 1) * 128]
                    nc.tensor.matmul(scA[:, ci * 128:(ci + 1) * 128], kst,
                                     qsA, start=True, stop=True)
                    nc.tensor.matmul(scB[:, ci * 128:(ci + 1) * 128], kst,
                                     qsB, start=True, stop=True)
                nc.tensor.matmul(scA[:, 640:768], kglobT[:, hp, :],
                                 qsA, start=True, stop=True)
                nc.tensor.matmul(scB[:, 640:768], kglobT[:, hp, :],
                                 qsB, start=True, stop=True)
                ets = []
                for sc, nm in ((scA, "A"), (scB, "B")):
                    et = mp.tile([128, 768], dt.bfloat16, tag="et", bufs=6,
                                 name=f"et{nm}")
                    nc.scalar.activation(et[:, :], sc[:, :], Act.Exp)
                    nc.vector.tensor_mul(et[:, 0:256], et[:, 0:256], mask2[:, :])
                    ets.append(et)
                return ets

            def emit_pv(t, p, etA, etB):
                # two 6-head accumulators: 6*65 floats = 390 <= 512 keeps every
                # per-head matmul output inside one PSUM bank
                if t not in pv_tiles:
                    pv_tiles[t] = (
                        pp.tile([128, H // 2, VW], dt.float32, tag="pv0",
                                bufs=1, name="ps_pv0"),
                        pp.tile([128, H // 2, VW], dt.float32, tag="pv1",
                                bufs=1, name="ps_pv1"),
                    )
                for et, h in ((etA, 2 * p), (etB, 2 * p + 1)):
                    pv_all = pv_tiles[t][h // (H // 2)]
                    hs = h % (H // 2)
                    n_ord = len(PV_ORDER)
                    for i, ci in enumerate(PV_ORDER):
                        if ci < 0:
                            nc.tensor.matmul(
                                pv_all[:, hs, :], et[:, 640:768],
                                vglob[:, h * VW:(h + 1) * VW],
                                start=(i == 0), stop=(i == n_ord - 1))
                        else:
                            src = PERM[ci]
                            nc.tensor.matmul(
                                pv_all[:, hs, :],
                                et[:, ci * 128:(ci + 1) * 128],
                                vext[:, t + src, h * VW:(h + 1) * VW],
                                start=(i == 0), stop=(i == n_ord - 1))

            def emit_normalize_pair(t, p):
                # per-pair normalize, all on vector: keeps scalar exp-only
                # (avoids queue saturation) and frees the single-buffered pv
                # tiles incrementally so the next tile's PV never stalls
                pv_all = pv_tiles[t][(2 * p) // (H // 2)]
                hs0 = (2 * p) % (H // 2)
                rec = mp.tile([128, 2, 1], dt.float32, tag="rec", bufs=4,
                              name="rec")
                nc.vector.reciprocal(rec[:, :, :],
                                     pv_all[:, hs0:hs0 + 2, HD:HD + 1])
                if t not in attn_tiles:
                    attn_tiles[t] = mp.tile([128, H, HD], dt.float32,
                                            tag="attn", bufs=2, name="attn_sb")
                attn_sb = attn_tiles[t]
                for j in range(2):
                    nc.vector.tensor_scalar_mul(attn_sb[:, 2 * p + j, :],
                                                pv_all[:, hs0 + j, 0:HD],
                                                rec[:, j:j + 1, 0:1])
                if p == H // 2 - 1:
                    pv_tiles.pop(t)
                    attn_tiles.pop(t)
                    nc.sync.dma_start(out_band[t * 128:(t + 1) * 128, :],
                                      attn_sb[:, :, :])

            jobs = [(t, p) for t in range(NQT) for p in range(H // 2)]
            pending = None
            for t, p in jobs:
                ets = emit_scores(t, p)
                if pending is not None:
                    pt, pp_, petA, petB = pending
                    emit_pv(pt, pp_, petA, petB)
                    emit_normalize_pair(pt, pp_)
                pending = (t, p, ets[0], ets[1])
            pt, pp_, petA, petB = pending
            emit_pv(pt, pp_, petA, petB)
            emit_normalize_pair(pt, pp_)

            # ---- phase 3: global-token partials over this core's keys ------
            # Per head: all 8 key-chunk score matmuls land in one [128, 512]
            # PSUM tile -> one Exp -> 8 PV accumulation matmuls.  Scores of
            # head h+1 are emitted before the PV of head h.
            gp_sb = mp.tile([G, H, VW], dt.float32)

            def emit_g_scores(hpair):
                hp = hpair
                sc3A = pp.tile([128, 512], dt.float32, tag="sc", bufs=3,
                               name="ps_sc3A")
                sc3B = pp.tile([128, 512], dt.float32, tag="sc", bufs=3,
                               name="ps_sc3B")
                for c in range(NGC):
                    kst = kgT[:, hp, c * 128:(c + 1) * 128]
                    nc.tensor.matmul(sc3A[:, c * 64:(c + 1) * 64], kst,
                                     qgTzA[:, hp, :], start=True, stop=True)
                    nc.tensor.matmul(sc3B[:, c * 64:(c + 1) * 64], kst,
                                     qgTzB[:, hp, :], start=True, stop=True)
                egs = []
                for sc3, nm in ((sc3A, "A"), (sc3B, "B")):
                    eg = mp.tile([128, 512], dt.bfloat16, tag="eg", bufs=4,
                                 name=f"eg{nm}")
                    nc.scalar.activation(eg[:, :], sc3[:, :], Act.Exp)
                    egs.append(eg)
                return egs

            def emit_g_pv(h, eg):
                pg = pp.tile([G, VW], dt.float32, tag="pv0", bufs=1, name="ps_pg")
                for c in range(NGC):
                    nc.tensor.matmul(pg[:, :], eg[:, c * 64:(c + 1) * 64],
                                     vgext[:, c, h * VW:(h + 1) * VW],
                                     start=(c == 0), stop=(c == NGC - 1))
                nc.vector.tensor_copy(gp_sb[:, h, :], pg[:, :])

            g_pending = None
            for hpair in range(H // 2):
                egA, egB = emit_g_scores(hpair)
                if g_pending is not None:
                    for h_, eg_ in g_pending:
                        emit_g_pv(h_, eg_)
                g_pending = ((2 * hpair, egA), (2 * hpair + 1, egB))
            for h_, eg_ in g_pending:
                emit_g_pv(h_, eg_)
            nc.sync.dma_start(gpart[:, :, :], gp_sb[:, :, :])

    nc.compile()
    return nc


def _get_program():
    global _PROGRAM
    if _PROGRAM is None:
        _PROGRAM = _build_program()
    return _PROGRAM


def kernel(hidden_states, Wq, bq, Wk, bk, Wv, bv, Wqg, bqg, Wkg, bkg, Wvg, bvg,
           attention_mask, n_global):
    from concourse.bass_utils import run_bass_kernel_spmd

    x = np.asarray(hidden_states, np.float32)
    am = np.asarray(attention_mask)
    ng = int(n_global)
    assert ng == G, f"kernel specialized for n_global={G}, got {ng}"
    scale = np.float32(1.0 / np.sqrt(HD))

    def chunked(a):
        # [E, N] -> [128, KE, N] matching the on-chip tile layout
        return np.ascontiguousarray(
            np.asarray(a).reshape(KE, 128, -1).transpose(1, 0, 2))

    wT = {
        "wqt": chunked((np.asarray(Wq, np.float32).T * scale).astype(BF16)),
        "wkt": chunked(np.asarray(Wk, np.float32).T.astype(BF16)),
        "wvt": chunked(np.asarray(Wv, np.float32).T.astype(BF16)),
        "wkgt": chunked(np.asarray(Wkg, np.float32).T.astype(BF16)),
        "wvgt": chunked(np.asarray(Wvg, np.float32).T.astype(BF16)),
        "wqgt": chunked((np.asarray(Wqg, np.float32).T * scale).astype(BF16)),
    }
    for bias in (bq, bk, bv, bqg, bkg, bvg):
        assert not np.any(np.asarray(bias)), "nonzero biases unsupported"

    tril = np.tril(np.ones((128, 128), np.float32))
    triu = np.triu(np.ones((128, 128), np.float32))
    mask2 = np.concatenate([tril, triu], axis=1).astype(BF16)

    xTg_pad = np.zeros((E, GP), np.float32)

    in_maps = []
    for core in range(NCORES):
        b, cb = divmod(core, 4)
        r0 = cb * CHUNK
        lo, hi = r0 - W, r0 + CHUNK + W
        slab = np.zeros((SLAB, E), np.float32)
        s0, s1 = max(0, lo), min(S, hi)
        slab[s0 - lo: s1 - lo] = x[b, s0:s1]
        valid = np.zeros(SLAB, np.float32)
        arange = np.arange(lo, hi)
        inb = (arange >= 0) & (arange < S)
        valid[inb] = (am[b, arange[inb]] == 0).astype(np.float32)
        gvalid = np.ones(G, np.float32)  # keys < n_global (ng == G)

        xtg = xTg_pad.copy()
        xtg[:, :G] = x[b, :G].T

        in_maps.append({
            "xT": chunked(np.ascontiguousarray(slab.T).astype(BF16)),
            "xTg": chunked(xtg.astype(BF16)),
            **wT,
            "vmask": np.ascontiguousarray(valid.reshape(NSC, 128).T),
            "vones": np.ascontiguousarray(
                np.repeat(valid.reshape(NSC, 128).T[:, :, None], H, axis=2)
            ).astype(BF16),
            "gmask": gvalid[:, None].copy(),
            "gones": np.repeat(gvalid[:, None], H, axis=1).astype(BF16),
            "mask2": mask2,
        })

    nc = _get_program()
    globals()['_last_in_maps'] = in_maps
    res = run_bass_kernel_spmd(nc, in_maps, core_ids=list(range(NCORES)))

    out = np.empty((B, S, E), np.float32)
    for core in range(NCORES):
        b, cb = divmod(core, 4)
        out[b, cb * CHUNK:(cb + 1) * CHUNK] = res.results[core]["out_band"]
    for b in range(B):
        acc = sum(res.results[b * 4 + cb]["gpart"] for cb in range(4))
        gout = acc[:, :, 0:HD] / acc[:, :, HD:HD + 1]
        out[b, :G] = gout.reshape(G, E)
    return out


# revision 23
# speedup vs baseline: 1.1085x; 1.0184x over previous
"""Longformer self-attention Bass kernel for 8 Trainium2 NeuronCores.

Sharding: data-parallel over batch (2) x sequence-parallel (4 chunks of 1024
queries). Each core receives a transposed x slab covering its rows +-W halo
(zero padded at sequence edges), the 64 global rows, and the full weight set.

On-device layout choices:
  - "T layout" [E_out on partitions, seq on free] for q/k/kg/qg so the banded
    score matmuls need no transposes; head pairs share 128-partition slices.
  - Scores are computed keys-on-partitions (S^T). Softmax runs without max
    subtraction (scores are O(1)); the denominator comes for free from a
    ones-column appended to v, so P@V and sum(exp) are one PSUM accumulation.
  - The global keys are zero-padded to 128 so their scores live in the same
    PSUM tile as the band ([128, 768] per head: 5 band chunks + glob block)
    and one Exp covers everything; padded keys get exp(0)=1 but multiply
    zeroed v/ones columns, contributing nothing.
  - Band-edge masking: score chunk order is [c0, c4, c1, c2, c3] so the two
    masked chunks are adjacent and one [128,256] mask multiply per head
    handles the window geometry.
  - Software pipelining: scores for head-pair p+1 are emitted before the PV
    matmuls of pair p, hiding the exp/mask latency behind PE work.
  - Stationary reuse: the moving q operands are kept in two zero-padded
    copies (head-A rows / head-B rows), so both heads of a pair contract
    against the same full-128-row kT stationary and the PE skips half the
    weight loads.  Same trick for the phase-3 global-token scores.
  - PSUM is managed at bank granularity (2KB, and a matmul output must not
    cross a bank): 3 score buffers (2 banks each) + two single-bank 6-head
    PV accumulators.  Normalize runs per-pair on Vector only (reciprocal +
    2 scale muls), keeping Scalar exp-only and freeing the single-buffered
    PV tiles incrementally.
  - All masking is data-driven (uniform SPMD program): key validity (padding
    + global-key exclusion from the band) is folded into the v/ones columns.
"""

import sys
import numpy as np

if '/opt/trn_rl_repo' not in sys.path:
    sys.path.insert(0, '/opt/trn_rl_repo')

import ml_dtypes

B, S, E, H, HD, G, W = 2, 4096, 768, 12, 64, 64, 256
NCORES = 8
CHUNK = 1024          # query rows per core
SLAB = 1536           # halo slab rows per core (CHUNK + 2W)
NQT = CHUNK // 128    # query tiles per core
NSC = SLAB // 128     # slab chunks of 128 keys
NGC = CHUNK // 128    # key chunks for the global-token partials
KE = E // 128         # contraction chunks over E
VW = HD + 1           # v head width incl. ones column
GP = 128              # global keys padded to 128
BF16 = ml_dtypes.bfloat16

# score-tile column order: key chunks [0, 4, 1, 2, 3]; masked chunks first.
PERM = (0, 4, 1, 2, 3)
# PV consumption order: unmasked middle chunks, then glob, then masked edges
PV_ORDER = (2, 3, 4, -1, 0, 1)   # -1 = glob block (cols 640:768)

_PROGRAM = None


def _build_program():
    import concourse.bass as bass
    import concourse.mybir as mybir
    import concourse.tile as tile
    from concourse import bacc

    dt = mybir.dt
    Act = mybir.ActivationFunctionType

    nc = bacc.Bacc("TRN2", target_bir_lowering=False, debug=False,
                   num_devices=NCORES)

    xT = nc.dram_tensor("xT", [128, KE, SLAB], dt.bfloat16, kind="ExternalInput")
    xTg = nc.dram_tensor("xTg", [128, KE, GP], dt.bfloat16, kind="ExternalInput")
    wts = {}
    for nm in ("wqt", "wkt", "wvt", "wkgt", "wvgt", "wqgt"):
        wts[nm] = nc.dram_tensor(nm, [128, KE, E], dt.bfloat16, kind="ExternalInput")
    vmask_d = nc.dram_tensor("vmask", [128, NSC], dt.float32, kind="ExternalInput")
    vones_d = nc.dram_tensor("vones", [128, NSC, H], dt.bfloat16, kind="ExternalInput")
    gmask_d = nc.dram_tensor("gmask", [G, 1], dt.float32, kind="ExternalInput")
    gones_d = nc.dram_tensor("gones", [G, H], dt.bfloat16, kind="ExternalInput")
    mask2_d = nc.dram_tensor("mask2", [128, 256], dt.bfloat16, kind="ExternalInput")
    out_band = nc.dram_tensor("out_band", [CHUNK, E], dt.float32, kind="ExternalOutput")
    gpart = nc.dram_tensor("gpart", [G, H, VW], dt.float32, kind="ExternalOutput")

    with tile.TileContext(nc) as tc:
        with tc.tile_pool(name="main", bufs=1) as mp, \
             tc.tile_pool(name="psum", bufs=1, space="PSUM") as pp:

            # ---- resident SBUF tensors -------------------------------------
            xT_sb = mp.tile([128, KE, SLAB], dt.bfloat16)
            xTg_sb = mp.tile([128, KE, GP], dt.bfloat16)
            w_sb = {nm: mp.tile([128, KE, E], dt.bfloat16, name=f"w_{nm}")
                    for nm in wts}
            qTzA = mp.tile([128, KE, CHUNK], dt.bfloat16, name="qTzA")
            qTzB = mp.tile([128, KE, CHUNK], dt.bfloat16, name="qTzB")
            kT = mp.tile([128, KE, SLAB], dt.bfloat16)
            kgT = mp.tile([128, KE, CHUNK], dt.bfloat16)
            kglobT = mp.tile([128, KE, GP], dt.bfloat16)
            qgTzA = mp.tile([128, KE, G], dt.bfloat16, name="qgTzA")
            qgTzB = mp.tile([128, KE, G], dt.bfloat16, name="qgTzB")
            vext = mp.tile([128, NSC, H * VW], dt.bfloat16)
            vgext = mp.tile([128, NGC, H * VW], dt.bfloat16)
            vglob = mp.tile([GP, H * VW], dt.bfloat16)
            vmask = mp.tile([128, NSC], dt.float32)
            gmask = mp.tile([G, 1], dt.float32)
            mask2 = mp.tile([128, 256], dt.bfloat16)

            # per-k-chunk DMAs so the first projection matmuls start as
            # soon as chunk 0 of x and wq arrives instead of waiting for the
            # whole 12.8 MB input load
            for k in range(KE):
                nc.sync.dma_start(xT_sb[:, k, :], xT[:, k, :])
                nc.sync.dma_start(w_sb["wqt"][:, k, :], wts["wqt"][:, k, :])
            for k in range(KE):
                nc.sync.dma_start(w_sb["wkt"][:, k, :], wts["wkt"][:, k, :])
            nc.sync.dma_start(xTg_sb[:, :, :], xTg[:, :, :])
            nc.sync.dma_start(vmask[:, :], vmask_d[:, :])
            nc.sync.dma_start(gmask[:, :], gmask_d[:, :])
            nc.sync.dma_start(mask2[:, :], mask2_d[:, :])
            # vglob rows G:GP stay zero (padded global keys kill their own
            # exp(0)=1 contributions); zero everything, then fill rows 0:G.
            nc.vector.memset(vglob[:, :], 0.0)
            nc.gpsimd.memset(qTzA[64:128, :, :], 0.0)
            nc.gpsimd.memset(qTzB[0:64, :, :], 0.0)
            nc.gpsimd.memset(qgTzA[64:128, :, :], 0.0)
            nc.gpsimd.memset(qgTzB[0:64, :, :], 0.0)
            # ones columns (gated by key validity)
            nc.sync.dma_start(
                vext.rearrange("p c (h e) -> p c h e", e=VW)[:, :, :, HD:HD + 1],
                vones_d.rearrange("p c (h o) -> p c h o", o=1))
            nc.sync.dma_start(
                vglob.rearrange("p (h e) -> p h e", e=VW)[0:G, :, HD:HD + 1],
                gones_d.rearrange("p (h o) -> p h o", o=1))
            nc.vector.memset(
                vgext.rearrange("p c (h e) -> p c h e", e=VW)[:, :, :, HD:HD + 1], 1.0)

            # ---- phase 1: projections --------------------------------------
            def proj_T(dst, wname, src, src_off, n_total, bias=None):
                # dst[e_out, n] = sum_e w[e, e_out] * src[e, src_off + n]
                for m in range(KE):
                    n0 = 0
                    while n0 < n_total:
                        nw = min(512, n_total - n0)
                        ps = pp.tile([128, 512], dt.float32, tag="sc", bufs=3,
                                     name="ps_projT")
                        for k in range(KE):
                            nc.tensor.matmul(
                                ps[:, :nw],
                                w_sb[wname][:, k, m * 128:(m + 1) * 128],
                                src[:, k, src_off + n0: src_off + n0 + nw],
                                start=(k == 0), stop=(k == KE - 1))
                        nc.scalar.activation(dst[:, m, n0:n0 + nw], ps[:, :nw],
                                             Act.Copy)
                        n0 += nw

            def proj_q():
                # like proj_T but splits each 128-row block between the two
                # zero-padded q tensors (head 2m rows 0:64 -> qTzA, head 2m+1
                # rows 64:128 -> qTzB) so score matmuls of a head pair can
                # share one full-128-row kT stationary
                for m in range(KE):
                    n0 = 0
                    while n0 < CHUNK:
                        nw = min(512, CHUNK - n0)
                        ps = pp.tile([128, 512], dt.float32, tag="sc", bufs=3,
                                     name="ps_projq")
                        for k in range(KE):
                            nc.tensor.matmul(
                                ps[:, :nw],
                                w_sb["wqt"][:, k, m * 128:(m + 1) * 128],
                                xT_sb[:, k, W + n0: W + n0 + nw],
                                start=(k == 0), stop=(k == KE - 1))
                        nc.scalar.activation(qTzA[0:64, m, n0:n0 + nw],
                                             ps[0:64, :nw], Act.Copy)
                        nc.scalar.activation(qTzB[64:128, m, n0:n0 + nw],
                                             ps[64:128, :nw], Act.Copy)
                        n0 += nw

            proj_q()
            for nm in ("wvt", "wkgt", "wvgt", "wqgt"):
                nc.sync.dma_start(w_sb[nm][:, :, :], wts[nm][:, :, :])
            proj_T(kT, "wkt", xT_sb, 0, SLAB)
            proj_T(kglobT, "wkt", xTg_sb, 0, GP)

            def proj_V(dst, wname, src, src_off, n_chunks, mask_tile):
                # dst[seq, h, d] = sum_e src[e, seq] * w[e, 64h+d], * mask[seq]
                for sI in range(n_chunks):
                    for h0, nw in ((0, 512), (512, 256)):
                        ps = pp.tile([128, 512], dt.float32, tag="sc", bufs=3,
                                     name="ps_projV")
                        for k in range(KE):
                            nc.tensor.matmul(
                                ps[:, :nw],
                                src[:, k, src_off + sI * 128: src_off + (sI + 1) * 128],
                                w_sb[wname][:, k, h0:h0 + nw],
                                start=(k == 0), stop=(k == KE - 1))
                        dv = dst.rearrange("p c (h e) -> p c h e", e=VW)[
                            :, sI, h0 // HD:(h0 + nw) // HD, 0:HD]
                        pv = ps[:, :nw].rearrange("p (h e) -> p h e", e=HD)
                        if mask_tile is None:
                            nc.vector.tensor_copy(dv, pv)
                        else:
                            nc.vector.tensor_scalar_mul(dv, pv,
                                                        mask_tile[:, sI:sI + 1])

            # vext v-data is gated by per-key validity (padding + global-key
            # exclusion from the band); vgext is not masked (global-token
            # attention sees every key).
            proj_V(vext, "wvt", xT_sb, 0, NSC, vmask)

            # v of the global rows (for the global-key part of the band
            # softmax), gated by j < n_global
            for h0, nw in ((0, 512), (512, 256)):
                psg = pp.tile([128, 512], dt.float32, tag="sc", bufs=3,
                              name="ps_vglob")
                for k in range(KE):
                    nc.tensor.matmul(
                        psg[0:G, :nw], xTg_sb[:, k, 0:G], w_sb["wvt"][:, k, h0:h0 + nw],
                        start=(k == 0), stop=(k == KE - 1))
                dv = vglob.rearrange("p (h e) -> p h e", e=VW)[
                    0:G, h0 // HD:(h0 + nw) // HD, 0:HD]
                nc.vector.tensor_scalar_mul(
                    dv, psg[0:G, :nw].rearrange("p (h e) -> p h e", e=HD),
                    gmask[:, :])

            # ---- phase 2: banded + global-key attention --------------------
            # Per head: one [128, 768] PSUM score tile (5 band chunks in PERM
            # order + padded glob block), one Exp, one [128,256] edge mask.
            # Scores of pair p+1 are emitted before the PV of pair p so the
            # exp/mask chain hides behind PE work.
            pv_tiles = {}      # t -> psum accumulator [128, H, VW]
            attn_tiles = {}

            def emit_scores(t, p):
                hp = p
                qsA = qTzA[:, hp, t * 128:(t + 1) * 128]
                qsB = qTzB[:, hp, t * 128:(t + 1) * 128]
                scA = pp.tile([128, 768], dt.float32, tag="sc", bufs=3,
                              name="ps_scA")
                scB = pp.tile([128, 768], dt.float32, tag="sc", bufs=3,
                              name="ps_scB")
                # full-128-row stationary (both heads' k dims); the moving q
                # is zero-padded on the other head's rows, so each head's
                # scores are exact and consecutive A/B matmuls reuse the
                # loaded weights
                for ci, src in enumerate(PERM):
                    kst = kT[:, hp, (t + src) * 128:(t + src + 1) * 128]
                    nc.tensor.matmul(scA[:, ci * 128:(ci + 1) * 128], kst,
                                     qsA, start=True, stop=True)
                    nc.tensor.matmul(scB[:, ci * 128:(ci + 1) * 128], kst,
                                     qsB, start=True, stop=True)
                nc.tensor.matmul(scA[:, 640:768], kglobT[:, hp, :],
                                 qsA, start=True, stop=True)
                nc.tensor.matmul(scB[:, 640:768], kglobT[:, hp, :],
                                 qsB, start=True, stop=True)
                ets = []
                for sc, nm in ((scA, "A"), (scB, "B")):
                    et = mp.tile([128, 768], dt.bfloat16, tag="et", bufs=6,
                                 name=f"et{nm}")
                    nc.scalar.activation(et[:, :], sc[:, :], Act.Exp)
                    nc.vector.tensor_mul(et[:, 0:256], et[:, 0:256], mask2[:, :])
                    ets.append(et)
                return ets

            def emit_pv(t, p, etA, etB):
                # two 6-head accumulators: 6*65 floats = 390 <= 512 keeps every
                # per-head matmul output inside one PSUM bank
                if t not in pv_tiles:
                    pv_tiles[t] = (
                        pp.tile([128, H // 2, VW], dt.float32, tag="pv0",
                                bufs=1, name="ps_pv0"),
                        pp.tile([128, H // 2, VW], dt.float32, tag="pv1",
                                bufs=1, name="ps_pv1"),
                    )
                for et, h in ((etA, 2 * p), (etB, 2 * p + 1)):
                    pv_all = pv_tiles[t][h // (H // 2)]
                    hs = h % (H // 2)
                    n_ord = len(PV_ORDER)
                    for i, ci in enumerate(PV_ORDER):
                        if ci < 0:
                            nc.tensor.matmul(
                                pv_all[:, hs, :], et[:, 640:768],
                                vglob[:, h * VW:(h + 1) * VW],
                                start=(i == 0), stop=(i == n_ord - 1))
                        else:
                            src = PERM[ci]
                            nc.tensor.matmul(
                                pv_all[:, hs, :],
                                et[:, ci * 128:(ci + 1) * 128],
                                vext[:, t + src, h * VW:(h + 1) * VW],
                                start=(i == 0), stop=(i == n_ord - 1))

            def emit_normalize_pair(t, p):
                # per-pair normalize, all on vector: keeps scalar exp-only
                # (avoids queue saturation) and frees the single-buffered pv
                # tiles incrementally so the next tile's PV never stalls
                pv_all = pv_tiles[t][(2 * p) // (H // 2)]
                hs0 = (2 * p) % (H // 2)
                rec = mp.tile([128, 2, 1], dt.float32, tag="rec", bufs=4,
                              name="rec")
                nc.vector.reciprocal(rec[:, :, :],
                                     pv_all[:, hs0:hs0 + 2, HD:HD + 1])
                if t not in attn_tiles:
                    attn_tiles[t] = mp.tile([128, H, HD], dt.float32,
                                            tag="attn", bufs=2, name="attn_sb")
                attn_sb = attn_tiles[t]
                for j in range(2):
                    nc.vector.tensor_scalar_mul(attn_sb[:, 2 * p + j, :],
                                                pv_all[:, hs0 + j, 0:HD],
                                                rec[:, j:j + 1, 0:1])
                if p == H // 2 - 1:
                    pv_tiles.pop(t)
                    attn_tiles.pop(t)
                    nc.sync.dma_start(out_band[t * 128:(t + 1) * 128, :],
                                      attn_sb[:, :, :])

            jobs = [(t, p) for t in range(NQT) for p in range(H // 2)]
            pending = None
            for t, p in jobs:
                ets = emit_scores(t, p)
                if pending is not None:
                    pt, pp_, petA, petB = pending
                    emit_pv(pt, pp_, petA, petB)
                    emit_normalize_pair(pt, pp_)
                pending = (t, p, ets[0], ets[1])
            pt, pp_, petA, petB = pending
            emit_pv(pt, pp_, petA, petB)
            emit_normalize_pair(pt, pp_)

            proj_T(kgT, "wkgt", xT_sb, W, CHUNK)
            for m in range(KE):
                psq = pp.tile([128, 512], dt.float32, tag="sc", bufs=3,
                              name="ps_projqg")
                for k in range(KE):
                    nc.tensor.matmul(
                        psq[:, :G], w_sb["wqgt"][:, k, m * 128:(m + 1) * 128],
                        xTg_sb[:, k, 0:G], start=(k == 0), stop=(k == KE - 1))
                nc.scalar.activation(qgTzA[0:64, m, :], psq[0:64, :G], Act.Copy)
                nc.scalar.activation(qgTzB[64:128, m, :], psq[64:128, :G],
                                     Act.Copy)
            proj_V(vgext, "wvgt", xT_sb, W, NGC, None)

            # ---- phase 3: global-token partials over this core's keys ------
            # Per head: all 8 key-chunk score matmuls land in one [128, 512]
            # PSUM tile -> one Exp -> 8 PV accumulation matmuls.  Scores of
            # head h+1 are emitted before the PV of head h.
            gp_sb = mp.tile([G, H, VW], dt.float32)

            def emit_g_scores(hpair):
                hp = hpair
                sc3A = pp.tile([128, 512], dt.float32, tag="sc", bufs=3,
                               name="ps_sc3A")
                sc3B = pp.tile([128, 512], dt.float32, tag="sc", bufs=3,
                               name="ps_sc3B")
                for c in range(NGC):
                    kst = kgT[:, hp, c * 128:(c + 1) * 128]
                    nc.tensor.matmul(sc3A[:, c * 64:(c + 1) * 64], kst,
                                     qgTzA[:, hp, :], start=True, stop=True)
                    nc.tensor.matmul(sc3B[:, c * 64:(c + 1) * 64], kst,
                                     qgTzB[:, hp, :], start=True, stop=True)
                egs = []
                for sc3, nm in ((sc3A, "A"), (sc3B, "B")):
                    eg = mp.tile([128, 512], dt.bfloat16, tag="eg", bufs=4,
                                 name=f"eg{nm}")
                    nc.scalar.activation(eg[:, :], sc3[:, :], Act.Exp)
                    egs.append(eg)
                return egs

            def emit_g_pv(h, eg):
                pg = pp.tile([G, VW], dt.float32, tag="pv0", bufs=1, name="ps_pg")
                for c in range(NGC):
                    nc.tensor.matmul(pg[:, :], eg[:, c * 64:(c + 1) * 64],
                                     vgext[:, c, h * VW:(h + 1) * VW],
                                     start=(c == 0), stop=(c == NGC - 1))
                nc.vector.tensor_copy(gp_sb[:, h, :], pg[:, :])

            g_pending = None
            for hpair in range(H // 2):
                egA, egB = emit_g_scores(hpair)
                if g_pending is not None:
                    for h_, eg_ in g_pending:
                        emit_g_pv(h_, eg_)
                g_pending = ((2 * hpair, egA), (2 * hpair + 1, egB))
            for h_, eg_ in g_pending:
                emit_g_pv(h_, eg_)
            nc.sync.dma_start(gpart[:, :, :], gp_sb[:, :, :])

    nc.compile()
    return nc


def _get_program():
    global _PROGRAM
    if _PROGRAM is None:
        _PROGRAM = _build_program()
    return _PROGRAM


def kernel(hidden_states, Wq, bq, Wk, bk, Wv, bv, Wqg, bqg, Wkg, bkg, Wvg, bvg,
           attention_mask, n_global):
    from concourse.bass_utils import run_bass_kernel_spmd

    x = np.asarray(hidden_states, np.float32)
    am = np.asarray(attention_mask)
    ng = int(n_global)
    assert ng == G, f"kernel specialized for n_global={G}, got {ng}"
    scale = np.float32(1.0 / np.sqrt(HD))

    def chunked(a):
        # [E, N] -> [128, KE, N] matching the on-chip tile layout
        return np.ascontiguousarray(
            np.asarray(a).reshape(KE, 128, -1).transpose(1, 0, 2))

    wT = {
        "wqt": chunked((np.asarray(Wq, np.float32).T * scale).astype(BF16)),
        "wkt": chunked(np.asarray(Wk, np.float32).T.astype(BF16)),
        "wvt": chunked(np.asarray(Wv, np.float32).T.astype(BF16)),
        "wkgt": chunked(np.asarray(Wkg, np.float32).T.astype(BF16)),
        "wvgt": chunked(np.asarray(Wvg, np.float32).T.astype(BF16)),
        "wqgt": chunked((np.asarray(Wqg, np.float32).T * scale).astype(BF16)),
    }
    for bias in (bq, bk, bv, bqg, bkg, bvg):
        assert not np.any(np.asarray(bias)), "nonzero biases unsupported"

    tril = np.tril(np.ones((128, 128), np.float32))
    triu = np.triu(np.ones((128, 128), np.float32))
    mask2 = np.concatenate([tril, triu], axis=1).astype(BF16)

    xTg_pad = np.zeros((E, GP), np.float32)

    in_maps = []
    for core in range(NCORES):
        b, cb = divmod(core, 4)
        r0 = cb * CHUNK
        lo, hi = r0 - W, r0 + CHUNK + W
        slab = np.zeros((SLAB, E), np.float32)
        s0, s1 = max(0, lo), min(S, hi)
        slab[s0 - lo: s1 - lo] = x[b, s0:s1]
        valid = np.zeros(SLAB, np.float32)
        arange = np.arange(lo, hi)
        inb = (arange >= 0) & (arange < S)
        valid[inb] = (am[b, arange[inb]] == 0).astype(np.float32)
        gvalid = np.ones(G, np.float32)  # keys < n_global (ng == G)

        xtg = xTg_pad.copy()
        xtg[:, :G] = x[b, :G].T

        in_maps.append({
            "xT": chunked(np.ascontiguousarray(slab.T).astype(BF16)),
            "xTg": chunked(xtg.astype(BF16)),
            **wT,
            "vmask": np.ascontiguousarray(valid.reshape(NSC, 128).T),
            "vones": np.ascontiguousarray(
                np.repeat(valid.reshape(NSC, 128).T[:, :, None], H, axis=2)
            ).astype(BF16),
            "gmask": gvalid[:, None].copy(),
            "gones": np.repeat(gvalid[:, None], H, axis=1).astype(BF16),
            "mask2": mask2,
        })

    nc = _get_program()
    globals()['_last_in_maps'] = in_maps
    res = run_bass_kernel_spmd(nc, in_maps, core_ids=list(range(NCORES)))

    out = np.empty((B, S, E), np.float32)
    for core in range(NCORES):
        b, cb = divmod(core, 4)
        out[b, cb * CHUNK:(cb + 1) * CHUNK] = res.results[core]["out_band"]
    for b in range(B):
        acc = sum(res.results[b * 4 + cb]["gpart"] for cb in range(4))
        gout = acc[:, :, 0:HD] / acc[:, :, HD:HD + 1]
        out[b, :G] = gout.reshape(G, E)
    return out
